# revision 32
# baseline (speedup 1.0000x reference)
"""ASP layer (low-rank masked attention + residual layernorm) on 8 TRN2 cores.

Sharding: core c handles batch b = c // 2, query half h = c % 2.
Each core receives x/mask for its batch ROTATED so that its 1024 queries are
rows 0:1024 (keys are just permuted; softmax and delta are invariant to key
order). The device program is identical on all cores (SPMD); only data
differs.

Device math per core (N=2048 keys, Q=1024 queries, D=1024, R=64):
  QtKt = [U|V]^T @ x^T          (PE bf16, fp32 accum; x^T precomputed on host)
  Qt   = QtKt[0:64]   * (mask*s).T   (DVE; s = 1/sqrt(r_eff) folded on host)
  Kt   = QtKt[64:128] * mask.T       (DVE)
  St   = Kt_tile^T @ Qt         (PE; scores TRANSPOSED [k, q] so exp output
                                 is directly the delta stationary)
  Et   = exp(St - 3.5)          (ACT, psum -> sbuf FP8 e4m3; softmax and the
                                 rs-scaled LN are shift-invariant.)
  rs   = ones^T @ Et            (PE fp8 DoubleRow; softmax row sums)
  delta= Et^T @ x8              (PE fp8 DoubleRow, fp32 accum)
  z    = rs*x_q + delta         (DVE; LN is scale-invariant per token)
  out  = LN(z)                  (DVE moments + single ACT sqrt)
gamma/beta are applied on the host.

Schedule (v2): the whole kernel is one software-pipelined PE stream.
  - All input DMAs are issued up front on 4 HWDGE rings (SP/ACT/Pool/DVE)
    in consumption order: uv, mt, ident, x^T c0, c1, x8 kt0-3, c2, kt4-7,
    c3, kt8-15, xq. ~6.75MB of critical bytes => first delta possible ~17us.
  - Warmup spin sized to end when uv+mt+x^T c0 land (opens the PE clock
    gate and hides the initial DMA latency).
  - Projections interleave qc0 scores+rowsums; the first delta pair (qb0/1)
    runs chain-major right after the projections with qc1 scores + qc1
    rowsums + both rowsum transposes as fillers; pairs (2,3),(4,5),(6,7)
    then stream clean with 6 PSUM banks so only qb7's epilogue trails the
    final matmul.
  - Tail: the tile drain's semaphore waits are distributed across PE/ACT/SP
    in parallel (instead of ~11 serial NOPs on SP), DVE/Pool gate on a flag
    semaphore, and Pool alone resets+clears the tile semaphore range. No
    full engine barriers at the end.
"""

import os
import sys

sys.path.insert(0, "/opt/trn_rl_repo")

import numpy as np
import ml_dtypes

B, N, D, R = 4, 2048, 1024, 64
NCORES = 8
Q = N // 2          # queries per core
NQB = Q // 128      # query blocks per core
NKT = N // 128      # key tiles
NDT = D // 128      # d tiles
LN_EPS = 1e-5
WARMUP_MM = 65      # 256-col PE spin sized to end as the first x^T d-tile
                    # lands in SBUF (~12us; full-clock spin mm = ~107ns)
EXP_SHIFT = -3.5    # exp(s + EXP_SHIFT): keeps Et below the e4m3 max of 240

BF16 = ml_dtypes.bfloat16
FP8 = ml_dtypes.float8_e4m3

_CACHE = {}


def _split_waits(nc, max_waits=1):
    """walrus in this container rejects instructions carrying more than ~1
    sem-wait (e.g. Drain/CTRL and the XPOSE DMA encodings). Move excess waits
    onto injected same-engine nops that precede the instruction — engines are
    program-ordered, so semantics are unchanged."""
    from concourse import mybir

    n = 0
    for fn in nc.m.functions:
        for bb in fn.blocks:
            insts = bb.instructions
            new_list = []
            for inst in insts:
                si = inst.sync_info
                waits = list(si.on_wait) if si and si.on_wait else []
                if len(waits) > max_waits:
                    excess = waits[: -max_waits]
                    si.on_wait = waits[-max_waits:]
                    for w in excess:
                        nop = mybir.InstNoOp(name=f"I-wsplit-{n}", ins=[],
                                             outs=[])
                        n += 1
                        nop.engine = inst.engine
                        nop.sync_info = mybir.SyncInfo(on_wait=[w],
                                                       on_update=[])
                        nc.register_instruction(nop)
                        new_list.append(nop)
                new_list.append(inst)
            insts[:] = new_list


def _patch_tile_drain():
    import concourse.tile as tile
    from concourse import mybir
    from concourse.vector_clock import ScopedClock

    if getattr(tile.TileContext, "_drain_patched", False):
        return

    def _drain_and_barrier(self, tick_clock, wait_clock):
        nc = self.nc
        # Collect the full end-of-kernel wait set on the sync drain, then
        # redistribute it: one wait per NOP, round-robin across PE/ACT/SP so
        # the ~600ns-per-sem-check cost is paid in parallel instead of as a
        # serial chain on SP. DVE and Pool (whose end-of-NEFF semaphore-wipe
        # chunks contain the live tile sems) gate on a flag incremented by
        # the three waiting engines; Pool then resets the tile sem range for
        # re-execution. No full engine barriers.
        drain_inst = nc.sync.drain()
        wait_clock.add_sem_waits(
            drain_inst.ins, ScopedClock({None: tick_clock.global_clock})
        )
        assert self.sems is not None
        popped = nc._tile_sem_poison_stack.pop()
        assert popped is self._sem_poison

        si = drain_inst.ins.sync_info
        waits = list(si.on_wait) if si and si.on_wait else []
        si.on_wait = []

        sem_nums = sorted(
            s.num if hasattr(s, "num") else s
            for s in self.sems.allocated().values()
        )
        flag = nc.alloc_semaphore("tail_flag")

        wait_engines = [nc.tensor, nc.scalar, nc.sync]
        for i, w in enumerate(waits):
            eng = wait_engines[i % len(wait_engines)]
            nop = eng.nop()
            nop.ins.sync_info = mybir.SyncInfo(on_wait=[w], on_update=[])
        for eng in wait_engines:
            eng.sem_inc(flag, 1)
        nc.vector.wait_ge(flag, len(wait_engines))
        nc.gpsimd.wait_ge(flag, len(wait_engines))
        if sem_nums:
            lo, hi = min(sem_nums), max(sem_nums)
            rng = range(lo, max(hi, flag.num) + 1)
            nc.gpsimd.dma_reset(rng)
            nc.gpsimd.sem_clear(rng)
        _split_waits(nc)

    tile.TileContext._drain_and_barrier = _drain_and_barrier
    tile.TileContext._drain_patched = True


def build_program():
    import contextlib

    import concourse.bass as bass
    import concourse.tile as tile
    from concourse import mybir

    _patch_tile_drain()
    f32 = mybir.dt.float32
    bf16 = mybir.dt.bfloat16
    fp8 = mybir.dt.float8e4
    AF = mybir.ActivationFunctionType
    DR = mybir.MatmulPerfMode.DoubleRow

    nc = bass.Bass("TRN2", target_bir_lowering=False, debug=False,
                   num_devices=NCORES)

    # x8/xt/xq are stored PARTITION-MAJOR in dram (host pre-shuffle): dram
    # bytes for partition p are contiguous, so every DMA line is a 2-4KB
    # linear run instead of 1KB (descriptor-rate-bound DMA runs ~2-4x
    # faster per queue).
    x8_d = nc.dram_tensor("x8", [128, NKT, D], fp8, kind="ExternalInput").ap()
    xt_d = nc.dram_tensor("xt", [128, NDT, N], bf16,
                          kind="ExternalInput").ap()
    xq_d = nc.dram_tensor("xq", [128, NQB, D], bf16,
                          kind="ExternalInput").ap()
    mt_d = nc.dram_tensor("mt", [2 * R, N], bf16, kind="ExternalInput").ap()
    uv_d = nc.dram_tensor("uv", [128, NDT, 2 * R], bf16,
                          kind="ExternalInput").ap()
    id_d = nc.dram_tensor("ident", [128, 128], f32, kind="ExternalInput").ap()
    out_d = nc.dram_tensor("out", [Q, D], bf16, kind="ExternalOutput").ap()

    with tile.TileContext(nc) as tc:
        with contextlib.ExitStack() as ctx:
            const = ctx.enter_context(tc.tile_pool(name="const", bufs=1))
            eps_sb = const.tile([128, 1], f32)
            shift_sb = const.tile([128, 1], f32)
            ones_sb = const.tile([128, 2, 128], fp8)
            warm_sb = const.tile([128, 256], bf16)
            uv_sb = const.tile([128, NDT, 2 * R], bf16)
            xt_sb = const.tile([128, NDT, N], bf16)
            mt_sb = const.tile([2 * R, N], bf16)
            x8_sb = const.tile([128, NKT, D], fp8)
            xq_sb = const.tile([128, NQB, D], bf16)
            id_sb = const.tile([128, 128], f32)
            qt_sb = const.tile([R, Q], bf16)
            kt_sb = const.tile([R, N], bf16)
            # Et layout: [p, qc, t(=kt pair), h, 512] — h indexes the kt pair
            # so [:, qc, t] is a ready-made [128, 2, 512] DoubleRow operand
            et_sb = const.tile([128, 2, NKT // 2, 2, 512], fp8)
            # throwaway Square output (only its accum_out matters); same-
            # engine WAW ordering makes sharing one buffer safe
            sq_scr = const.tile([128, 512], f32)

            # warm_sb first: the PE warmup spin waits only on this memset
            nc.vector.memset(warm_sb, 0.5)
            nc.vector.memset(ones_sb, 1.0)
            nc.vector.memset(eps_sb, LN_EPS)
            nc.vector.memset(shift_sb, EXP_SHIFT)

            # ---- all input DMAs up front. Each ring is ONE serial hw
            # queue (measured ~160GB/s at 2KB lines, ~250-300 at 4KB), so
            # full-d-tile x^T transfers (4KB lines) are split across the
            # sync+scalar rings while gpsimd's software queue (~170GB/s)
            # takes mt and most of x8. uv goes first on sync: the whole
            # projection phase waits on it.
            def xt_full(ring, dt):   # x^T d-tile dt, all 2048 cols (512KB)
                ring.dma_start(out=xt_sb[:, dt, :], in_=xt_d[:, dt, :])

            def x8q(ring, q):    # x8 quad: key tiles 4q..4q+3 (512KB)
                ring.dma_start(out=x8_sb[:, 4 * q:4 * q + 4, :],
                               in_=x8_d[:, 4 * q:4 * q + 4, :])

            def xqp(ring, b):    # xq pair: query blocks b, b+1 (512KB)
                ring.dma_start(out=xq_sb[:, b:b + 2, :],
                               in_=xq_d[:, b:b + 2, :])

            # aggregate HBM read is the wall (~350GB/s across all queues),
            # so rings strictly prioritize: x^T (which gates everything)
            # split across all three, then x8 quads, then xq. Only x8q0
            # jumps the queue (gpsimd, early) so the first delta group
            # isn't gated on the whole x^T load finishing first.
            nc.sync.dma_start(out=uv_sb, in_=uv_d)
            for dt in (0, 3, 6):
                xt_full(nc.sync, dt)
            x8q(nc.sync, 1)
            xqp(nc.sync, 0)
            xqp(nc.sync, 2)
            for dt in (1, 4, 7):
                xt_full(nc.scalar, dt)
            x8q(nc.scalar, 2)
            xqp(nc.scalar, 4)
            xqp(nc.scalar, 6)
            nc.gpsimd.dma_start(out=mt_sb, in_=mt_d)
            x8q(nc.gpsimd, 0)
            for dt in (2, 5):
                xt_full(nc.gpsimd, dt)
            nc.gpsimd.dma_start(out=id_sb, in_=id_d)
            x8q(nc.gpsimd, 3)

            # ---- pools ----
            work = ctx.enter_context(tc.tile_pool(name="work", bufs=2))
            keep = ctx.enter_context(tc.tile_pool(name="keep", bufs=1))
            small = ctx.enter_context(tc.tile_pool(name="small", bufs=3))
            rsq_sb = keep.tile([128, NQB], f32)   # softmax rowsums, [q,1]/qb

            # PSUM budget (8 banks): phase 0: warm(1) + ps0(4) + st(1x2) +
            # rr(1) = 8; phase 1 (projections done): st(2) + rr(1) + d(5)
            # = 8; phase 2 (scores+rowsums done): d(6).
            phaseA = ctx.enter_context(contextlib.ExitStack())
            st_pool = phaseA.enter_context(
                tc.tile_pool(name="st_ps", bufs=1, space="PSUM"))
            rr_pool = phaseA.enter_context(
                tc.tile_pool(name="rr_ps", bufs=1, space="PSUM"))

            def st_pair(qc, t):
                """St = Kt_kt^T @ Qt_qc for kt pair (2t, 2t+1); Et = exp."""
                qlo = qc * 512
                st_ps = st_pool.tile([128, 2, 512], f32,
                                     name=f"st_{qc}_{t}", tag="st")
                for h in range(2):
                    kt = 2 * t + h
                    nc.tensor.matmul(
                        st_ps[:, h],
                        kt_sb[:, kt * 128:(kt + 1) * 128],
                        qt_sb[:, qlo:qlo + 512],
                        start=True, stop=True,
                    )
                nc.scalar.activation(out=et_sb[:, qc, t], in_=st_ps,
                                     func=AF.Exp, bias=shift_sb)

            def rs_mm(qc, t, rr_ps):
                """one accumulating DoubleRow step of rs = ones^T @ Et; the
                all-ones stationary is [128, 2, 128] (M=1 fails the walrus
                ldweights ISA check), so every psum partition receives the
                same rowsum row — rs_fix reads row 0."""
                nc.tensor.matmul(
                    rr_ps, ones_sb,
                    et_sb[:, qc, t],
                    start=(t == 0), stop=(t == NKT // 2 - 1),
                    perf_mode=DR,
                )

            def rs_fix(qc, rr_ps):
                """rowsums psum -> sbuf, then layout fix [1,q] -> [q,1] per
                query block via tiny PE transposes sharing the rr bank."""
                rs_sb = small.tile([1, 512], f32, tag="rs_sb")
                nc.vector.tensor_copy(rs_sb, rr_ps[0:1, :])
                for j in range(4):
                    qb = qc * 4 + j
                    nc.tensor.transpose(rr_ps[:, j:j + 1],
                                        rs_sb[0:1, j * 128:(j + 1) * 128],
                                        id_sb[0:1, 0:1])
                    nc.vector.tensor_copy(rsq_sb[:, qb:qb + 1],
                                          rr_ps[:, j:j + 1])

            def epi_half(qb, d_ps, dc, y, zs, zss):
                """z half: rs*x_q + delta for 512 features. sum(z) rides the
                same DVE op via accum_out; sum(z^2) goes to the idle ACT as
                Square+accum."""
                lo, hi = dc * 512, (dc + 1) * 512
                nc.vector.scalar_tensor_tensor(
                    out=y[:, lo:hi], in0=xq_sb[:, qb, lo:hi],
                    scalar=rsq_sb[:, qb:qb + 1], in1=d_ps,
                    op0=mybir.AluOpType.mult, op1=mybir.AluOpType.add,
                    accum_out=zs[:, dc:dc + 1],
                )
                nc.scalar.activation(out=sq_scr, in_=y[:, lo:hi],
                                     func=AF.Square,
                                     accum_out=zss[:, dc:dc + 1])

            def epi_finish(qb, y, zs, zss, ts_act=False):
                """out = LN(z) from the accumulated moments:
                var = (sum(z^2) - sum(z)^2/D) / D; out = z*rstd - mean*rstd.
                Split per dc half so each half's store DMA starts early."""
                t1 = small.tile([128, 1], f32, tag="t1")
                nc.vector.tensor_add(t1, zs[:, 0:1], zs[:, 1:2])
                dv = small.tile([128, 1], f32, tag="dv")
                nc.vector.scalar_tensor_tensor(
                    out=dv, in0=t1, scalar=1.0 / D, in1=t1,
                    op0=mybir.AluOpType.mult, op1=mybir.AluOpType.mult)
                t2 = small.tile([128, 1], f32, tag="t2")
                nc.vector.tensor_add(t2, zss[:, 0:1], zss[:, 1:2])
                vv = small.tile([128, 1], f32, tag="vv")
                nc.vector.tensor_sub(vv, t2, dv)
                sd = small.tile([128, 1], f32, tag="sd")
                nc.scalar.activation(out=sd, in_=vv, func=AF.Sqrt,
                                     scale=1.0 / D, bias=eps_sb)
                rstd = small.tile([128, 1], f32, tag="rstd")
                nc.vector.reciprocal(rstd, sd)
                nmr = small.tile([128, 1], f32, tag="nmr")
                nc.vector.scalar_tensor_tensor(
                    out=nmr, in0=t1, scalar=-1.0 / D, in1=rstd,
                    op0=mybir.AluOpType.mult, op1=mybir.AluOpType.mult)
                o_sb = work.tile([128, D], bf16, tag="o")
                # store halves on alternating rings; for the tail pair the
                # out-scale can run on ACT (Copy table is always resident)
                # so the last two epilogues don't serialize on DVE.
                store_rings = [nc.sync, nc.scalar]
                for dc in range(2):
                    lo, hi = dc * 512, (dc + 1) * 512
                    if ts_act:
                        nc.scalar.activation(out=o_sb[:, lo:hi],
                                             in_=y[:, lo:hi],
                                             func=AF.Identity,
                                             scale=rstd, bias=nmr)
                    else:
                        nc.vector.tensor_scalar(out=o_sb[:, lo:hi],
                                                in0=y[:, lo:hi],
                                                scalar1=rstd, scalar2=nmr,
                                                op0=mybir.AluOpType.mult,
                                                op1=mybir.AluOpType.add)
                    store_rings[dc].dma_start(
                        out=out_d[qb * 128:(qb + 1) * 128, lo:hi],
                        in_=o_sb[:, lo:hi])

            def make_chain(qb, dc, d_pool, y, zs, zss):
                """8 accumulating delta matmul thunks for one (qb, dc) half;
                the stop matmul issues the half's DVE/ACT epilogue inline."""
                qc, j = divmod(qb, 4)
                d_ps = d_pool.tile([128, 512], f32, name=f"d_{qb}_{dc}",
                                   tag="d")
                mms = []
                for t in range(NKT // 2):
                    def mm(t=t):
                        nc.tensor.matmul(
                            d_ps,
                            et_sb[:, qc, t, :, j * 128:(j + 1) * 128],
                            x8_sb[:, 2 * t:2 * t + 2,
                                  dc * 512:(dc + 1) * 512],
                            start=(t == 0), stop=(t == NKT // 2 - 1),
                            perf_mode=DR,
                        )
                        if t == NKT // 2 - 1:
                            epi_half(qb, d_ps, dc, y, zs, zss)
                    mms.append(mm)
                return mms

            def qb_bufs(qb):
                y = work.tile([128, D], f32, tag="y")
                zs = small.tile([128, 2], f32, tag="zs")
                zss = small.tile([128, 2], f32, tag="zss")
                return y, zs, zss

            # ---- phase 0: warmup spin, then all four projection chunk
            # chains in d-tile lockstep. Each full-d-tile DMA arrival
            # unlocks 4 matmuls (one per chunk); the chains all stop at the
            # last arrival, which is the earliest Qt/Kt can exist anyway
            # (the contraction needs every d-tile). ----
            rr0 = rr_pool.tile([128, 512], f32, name="rr_0", tag="rr")
            with tc.tile_pool(name="warm", bufs=1, space="PSUM") as warm, \
                    tc.tile_pool(name="ps0", bufs=1, space="PSUM") as ps0:
                w_ps = warm.tile([128, 256], f32)
                for _ in range(WARMUP_MM):
                    nc.tensor.matmul(w_ps, warm_sb[:, 0:128], warm_sb,
                                     start=True, stop=True)
                qk = [ps0.tile([128, 512], f32, name=f"qk_{c}")
                      for c in range(4)]
                for i in range(NDT):
                    for c in range(4):
                        nc.tensor.matmul(
                            qk[c], uv_sb[:, i, :],
                            xt_sb[:, i, c * 512:(c + 1) * 512],
                            start=(i == 0), stop=(i == NDT - 1),
                        )
                for c in range(4):
                    lo, hi = c * 512, (c + 1) * 512
                    if lo < Q:
                        nc.vector.tensor_mul(qt_sb[:, lo:hi],
                                             qk[c][0:R, :], mt_sb[0:R, lo:hi])
                    nc.vector.tensor_mul(kt_sb[:, lo:hi],
                                         qk[c][R:2 * R, :],
                                         mt_sb[R:2 * R, lo:hi])

            # ---- phase 1: first delta pair (qb0, qb1) t-major — each Et
            # tile feeds 4 delta matmuls right as its exp lands (the exp
            # stream on ACT is the pacer here) — with all remaining score
            # pairs and the qc0 rowsum chain as fillers. ----
            d5 = phaseA.enter_context(
                tc.tile_pool(name="d_ps", bufs=5, space="PSUM"))
            st_pair(0, 0)
            st_pair(0, 1)
            y0, zs0, zss0 = qb_bufs(0)
            y1, zs1, zss1 = qb_bufs(1)
            a0 = make_chain(0, 0, d5, y0, zs0, zss0)
            a1 = make_chain(0, 1, d5, y0, zs0, zss0)
            b0 = make_chain(1, 0, d5, y1, zs1, zss1)
            b1 = make_chain(1, 1, d5, y1, zs1, zss1)
            rr1 = rr_pool.tile([128, 512], f32, name="rr_1", tag="rr")

            # one score pair per delta group (a second back-to-back pair
            # would stall on the single st psum buffer waiting for the
            # previous exp to drain)
            ST1 = [(0, 2), (0, 3), (0, 4), (0, 5), (0, 6), (0, 7),
                   (1, 0), (1, 1)]
            for t in range(NKT // 2):
                if t == NKT // 2 - 1:
                    rs_mm(0, 7, rr0)
                    rs_fix(0, rr0)
                st_pair(*ST1[t])
                for mm in (a0[t], a1[t], b0[t], b1[t]):
                    mm()
                if t < NKT // 2 - 1:
                    rs_mm(0, t, rr0)
            epi_finish(0, y0, zs0, zss0)
            epi_finish(1, y1, zs1, zss1)

            # ---- phase 2: pair (2,3) chain-major carrying the remaining
            # qc1 score pairs, the qc1 rowsum chain and its transpose ----
            y2, zs2, zss2 = qb_bufs(2)
            y3, zs3, zss3 = qb_bufs(3)
            c20 = make_chain(2, 0, d5, y2, zs2, zss2)
            c30 = make_chain(3, 0, d5, y3, zs3, zss3)
            c21 = make_chain(2, 1, d5, y2, zs2, zss2)
            c31 = make_chain(3, 1, d5, y3, zs3, zss3)
            ST2 = {0: (1, 2), 3: (1, 3), 6: (1, 4), 9: (1, 5), 12: (1, 6),
                   15: (1, 7)}
            RS2 = {2: 0, 5: 1, 8: 2, 11: 3, 14: 4, 17: 5, 20: 6, 23: 7}
            for i, mm in enumerate(c20 + c30 + c21):
                if i in ST2:
                    st_pair(*ST2[i])
                mm()
                if i in RS2:
                    rs_mm(1, RS2[i], rr1)
            rs_fix(1, rr1)
            epi_finish(2, y2, zs2, zss2)
            for mm in c31:
                mm()
            epi_finish(3, y3, zs3, zss3)

            # ---- phase 3: pairs (4,5) and (6,7), 6 PSUM banks, pure
            # deltas; the final pair's out-scales ride ACT so only one
            # short DVE chain trails the last matmul ----
            phaseA.close()
            d6 = ctx.enter_context(
                tc.tile_pool(name="d_ps_b", bufs=6, space="PSUM"))
            for qa in range(4, NQB, 2):
                tail = qa + 1 == NQB - 1
                ya, zsa, zssa = qb_bufs(qa)
                yb, zsb, zssb = qb_bufs(qa + 1)
                ca0 = make_chain(qa, 0, d6, ya, zsa, zssa)
                cb0 = make_chain(qa + 1, 0, d6, yb, zsb, zssb)
                for mm in ca0 + cb0:
                    mm()
                ca1 = make_chain(qa, 1, d6, ya, zsa, zssa)
                for mm in ca1:
                    mm()
                epi_finish(qa, ya, zsa, zssa, ts_act=tail)
                cb1 = make_chain(qa + 1, 1, d6, yb, zsb, zssb)
                for mm in cb1:
                    mm()
                epi_finish(qa + 1, yb, zsb, zssb)

    return nc


def prep_core_inputs(x, mask, U, V):
    """Per-core input dicts (host-side sharding/layout prep)."""
    # [D, 2R] -> [128, NDT, 2R]: partition-major so the device DMA is one
    # contiguous 2KB-per-partition read
    uv = np.ascontiguousarray(
        np.concatenate([U, V], axis=1).astype(BF16)
        .reshape(NDT, 128, 2 * R).transpose(1, 0, 2))
    ident = np.eye(128, dtype=np.float32)
    ins = []
    for c in range(NCORES):
        b, h = divmod(c, 2)
        rot = np.roll(np.arange(N), -h * Q)
        xr = np.ascontiguousarray(x[b][rot])            # [N, D] f32
        mr = np.ascontiguousarray(mask[b][rot])         # [N, R] f32
        s = 1.0 / np.sqrt(np.maximum(mr.sum(axis=1), 1.0))   # [N]
        mq = (mr * s[:, None]).astype(BF16).T           # [R, N]
        mk = mr.astype(BF16).T                          # [R, N]
        xbf = xr.astype(BF16)
        # partition-major dram layouts: [...] -> [128, tiles, free] so each
        # partition's dram bytes are one contiguous run (big DMA lines)
        x8p = np.ascontiguousarray(
            xr.astype(FP8).reshape(NKT, 128, D).transpose(1, 0, 2))
        xtp = np.ascontiguousarray(
            xbf.T.reshape(NDT, 128, N).transpose(1, 0, 2))
        xqp = np.ascontiguousarray(
            xbf[:Q].reshape(NQB, 128, D).transpose(1, 0, 2))
        ins.append({
            "x8": x8p,
            "xt": xtp,
            "xq": xqp,
            "mt": np.ascontiguousarray(np.concatenate([mq, mk], axis=0)),
            "uv": uv,
            "ident": ident,
        })
    return ins


WALRUS_MAX_SEM = 176    # the NEFF exit routine wipes semaphores 0..max in
                        # ~51-per-engine serial chunks (~6us); our program
                        # tops out at sem ~170, so cap the wipe there.


def _patch_walrus_maxsem():
    if not WALRUS_MAX_SEM:
        return
    import concourse.bass_utils as bu

    if getattr(bu, "_asp_walrus_shim", None):
        return
    real = bu.get_walrus_driver()
    shim = f"/tmp/asp_walrus_shim_{WALRUS_MAX_SEM}.sh"
    with open(shim, "w") as f:
        f.write(f'#!/bin/sh\nexec {real} "$@" '
                f'--max-sem-num={WALRUS_MAX_SEM}\n')
    os.chmod(shim, 0o755)
    bu.get_walrus_driver = lambda: shim
    bu._asp_walrus_shim = shim


def run_cores(ins, trace=False, trace_kwargs=None):
    from concourse.bass_utils import run_bass_kernel_spmd

    _patch_walrus_maxsem()
    if "nc" not in _CACHE:
        _CACHE["nc"] = build_program()
    kw = {}
    if trace:
        kw["trace"] = True
        kw.update(trace_kwargs or {})
    return run_bass_kernel_spmd(_CACHE["nc"], ins, list(range(NCORES)), **kw)


def kernel(x, mask, U, V, gamma, beta):
    x = np.asarray(x, dtype=np.float32)
    mask = np.asarray(mask, dtype=np.float32)
    U = np.asarray(U, dtype=np.float32)
    V = np.asarray(V, dtype=np.float32)
    gamma = np.asarray(gamma, dtype=np.float32)
    beta = np.asarray(beta, dtype=np.float32)

    ins = prep_core_inputs(x, mask, U, V)
    res = run_cores(ins)
    out = np.empty((B, N, D), dtype=np.float32)
    for c in range(NCORES):
        b, h = divmod(c, 2)
        out[b, h * Q:(h + 1) * Q] = res.results[c]["out"].astype(np.float32)
    return out * gamma + beta


# revision 35
# speedup vs baseline: 1.0279x; 1.0279x over previous
"""ASP layer (low-rank masked attention + residual layernorm) on 8 TRN2 cores.

Sharding: core c handles batch b = c // 2, query half h = c % 2.
Each core receives x/mask for its batch ROTATED so that its 1024 queries are
rows 0:1024 (keys are just permuted; softmax and delta are invariant to key
order). The device program is identical on all cores (SPMD); only data
differs.

Device math per core (N=2048 keys, Q=1024 queries, D=1024, R=64):
  QtKt = [U|V]^T @ x^T          (PE bf16, fp32 accum; x^T precomputed on host)
  Qt   = QtKt[0:64]   * (mask*s).T   (DVE; s = 1/sqrt(r_eff) folded on host)
  Kt   = QtKt[64:128] * mask.T       (DVE)
  St   = Kt_tile^T @ Qt         (PE; scores TRANSPOSED [k, q] so exp output
                                 is directly the delta stationary)
  Et   = exp(St - 3.5)          (ACT, psum -> sbuf FP8 e4m3; softmax and the
                                 rs-scaled LN are shift-invariant.)
  rs   = ones^T @ Et            (PE fp8 DoubleRow; softmax row sums)
  delta= Et^T @ x8              (PE fp8 DoubleRow, fp32 accum)
  z    = rs*x_q + delta         (DVE; LN is scale-invariant per token)
  out  = LN(z)                  (DVE moments + single ACT sqrt)
gamma/beta are applied on the host.

Schedule (v2): the whole kernel is one software-pipelined PE stream.
  - All input DMAs are issued up front on 4 HWDGE rings (SP/ACT/Pool/DVE)
    in consumption order: uv, mt, ident, x^T c0, c1, x8 kt0-3, c2, kt4-7,
    c3, kt8-15, xq. ~6.75MB of critical bytes => first delta possible ~17us.
  - Warmup spin sized to end when uv+mt+x^T c0 land (opens the PE clock
    gate and hides the initial DMA latency).
  - Projections interleave qc0 scores+rowsums; the first delta pair (qb0/1)
    runs chain-major right after the projections with qc1 scores + qc1
    rowsums + both rowsum transposes as fillers; pairs (2,3),(4,5),(6,7)
    then stream clean with 6 PSUM banks so only qb7's epilogue trails the
    final matmul.
  - Tail: the tile drain's semaphore waits are distributed across PE/ACT/SP
    in parallel (instead of ~11 serial NOPs on SP), DVE/Pool gate on a flag
    semaphore, and Pool alone resets+clears the tile semaphore range. No
    full engine barriers at the end.
"""

import os
import sys

sys.path.insert(0, "/opt/trn_rl_repo")

import numpy as np
import ml_dtypes

B, N, D, R = 4, 2048, 1024, 64
NCORES = 8
Q = N // 2          # queries per core
NQB = Q // 128      # query blocks per core
NKT = N // 128      # key tiles
NDT = D // 128      # d tiles
LN_EPS = 1e-5
WARMUP_MM = 65      # 256-col PE spin sized to end as the first x^T d-tile
                    # lands in SBUF (~12us; full-clock spin mm = ~107ns)
EXP_SHIFT = -3.5    # exp(s + EXP_SHIFT): keeps Et below the e4m3 max of 240

BF16 = ml_dtypes.bfloat16
FP8 = ml_dtypes.float8_e4m3

_CACHE = {}


def _split_waits(nc, max_waits=1):
    """walrus in this container rejects instructions carrying more than ~1
    sem-wait (e.g. Drain/CTRL and the XPOSE DMA encodings). Move excess waits
    onto injected same-engine nops that precede the instruction — engines are
    program-ordered, so semantics are unchanged."""
    from concourse import mybir

    n = 0
    for fn in nc.m.functions:
        for bb in fn.blocks:
            insts = bb.instructions
            new_list = []
            for inst in insts:
                si = inst.sync_info
                waits = list(si.on_wait) if si and si.on_wait else []
                if len(waits) > max_waits:
                    excess = waits[: -max_waits]
                    si.on_wait = waits[-max_waits:]
                    for w in excess:
                        nop = mybir.InstNoOp(name=f"I-wsplit-{n}", ins=[],
                                             outs=[])
                        n += 1
                        nop.engine = inst.engine
                        nop.sync_info = mybir.SyncInfo(on_wait=[w],
                                                       on_update=[])
                        nc.register_instruction(nop)
                        new_list.append(nop)
                new_list.append(inst)
            insts[:] = new_list


def _patch_tile_drain():
    import concourse.tile as tile
    from concourse import mybir
    from concourse.vector_clock import ScopedClock

    if getattr(tile.TileContext, "_drain_patched", False):
        return

    def _drain_and_barrier(self, tick_clock, wait_clock):
        nc = self.nc
        # Collect the full end-of-kernel wait set on the sync drain, then
        # redistribute it: one wait per NOP, round-robin across PE/ACT/SP so
        # the ~600ns-per-sem-check cost is paid in parallel instead of as a
        # serial chain on SP. DVE and Pool (whose end-of-NEFF semaphore-wipe
        # chunks contain the live tile sems) gate on a flag incremented by
        # the three waiting engines; Pool then resets the tile sem range for
        # re-execution. No full engine barriers.
        drain_inst = nc.sync.drain()
        wait_clock.add_sem_waits(
            drain_inst.ins, ScopedClock({None: tick_clock.global_clock})
        )
        assert self.sems is not None
        popped = nc._tile_sem_poison_stack.pop()
        assert popped is self._sem_poison

        si = drain_inst.ins.sync_info
        waits = list(si.on_wait) if si and si.on_wait else []
        si.on_wait = []

        sem_nums = sorted(
            s.num if hasattr(s, "num") else s
            for s in self.sems.allocated().values()
        )
        flag = nc.alloc_semaphore("tail_flag")

        wait_engines = [nc.tensor, nc.scalar, nc.sync]
        for i, w in enumerate(waits):
            eng = wait_engines[i % len(wait_engines)]
            nop = eng.nop()
            nop.ins.sync_info = mybir.SyncInfo(on_wait=[w], on_update=[])
        for eng in wait_engines:
            eng.sem_inc(flag, 1)
        nc.vector.wait_ge(flag, len(wait_engines))
        nc.gpsimd.wait_ge(flag, len(wait_engines))
        if sem_nums:
            lo, hi = min(sem_nums), max(sem_nums)
            rng = range(lo, max(hi, flag.num) + 1)
            nc.gpsimd.dma_reset(rng)
            nc.gpsimd.sem_clear(rng)
        _split_waits(nc)

    tile.TileContext._drain_and_barrier = _drain_and_barrier
    tile.TileContext._drain_patched = True


def build_program():
    import contextlib

    import concourse.bass as bass
    import concourse.tile as tile
    from concourse import mybir

    _patch_tile_drain()
    f32 = mybir.dt.float32
    bf16 = mybir.dt.bfloat16
    fp8 = mybir.dt.float8e4
    AF = mybir.ActivationFunctionType
    DR = mybir.MatmulPerfMode.DoubleRow

    nc = bass.Bass("TRN2", target_bir_lowering=False, debug=False,
                   num_devices=NCORES)

    # x8/xt/xq are stored PARTITION-MAJOR in dram (host pre-shuffle): dram
    # bytes for partition p are contiguous, so every DMA line is a 2-4KB
    # linear run instead of 1KB (descriptor-rate-bound DMA runs ~2-4x
    # faster per queue).
    x8_d = nc.dram_tensor("x8", [128, NKT, D], fp8, kind="ExternalInput").ap()
    xt_d = nc.dram_tensor("xt", [128, NDT, N], bf16,
                          kind="ExternalInput").ap()
    xq_d = nc.dram_tensor("xq", [128, NQB, D], bf16,
                          kind="ExternalInput").ap()
    mt_d = nc.dram_tensor("mt", [2 * R, N], bf16, kind="ExternalInput").ap()
    uv_d = nc.dram_tensor("uv", [128, NDT, 2 * R], bf16,
                          kind="ExternalInput").ap()
    id_d = nc.dram_tensor("ident", [128, 128], f32, kind="ExternalInput").ap()
    out_d = nc.dram_tensor("out", [Q, D], bf16, kind="ExternalOutput").ap()

    with tile.TileContext(nc) as tc:
        with contextlib.ExitStack() as ctx:
            const = ctx.enter_context(tc.tile_pool(name="const", bufs=1))
            eps_sb = const.tile([128, 1], f32)
            shift_sb = const.tile([128, 1], f32)
            ones_sb = const.tile([128, 2, 128], fp8)
            warm_sb = const.tile([128, 256], bf16)
            uv_sb = const.tile([128, NDT, 2 * R], bf16)
            xt_sb = const.tile([128, NDT, N], bf16)
            mt_sb = const.tile([2 * R, N], bf16)
            x8_sb = const.tile([128, NKT, D], fp8)
            xq_sb = const.tile([128, NQB, D], bf16)
            id_sb = const.tile([128, 128], f32)
            qt_sb = const.tile([R, Q], bf16)
            kt_sb = const.tile([R, N], bf16)
            # Et layout: [p, qc, t(=kt pair), h, 512] — h indexes the kt pair
            # so [:, qc, t] is a ready-made [128, 2, 512] DoubleRow operand
            et_sb = const.tile([128, 2, NKT // 2, 2, 512], fp8)
            # throwaway Square output (only its accum_out matters); same-
            # engine WAW ordering makes sharing one buffer safe
            sq_scr = const.tile([128, 512], f32)

            # warm_sb first: the PE warmup spin waits only on this memset
            nc.vector.memset(warm_sb, 0.5)
            nc.vector.memset(ones_sb, 1.0)
            nc.vector.memset(eps_sb, LN_EPS)
            nc.vector.memset(shift_sb, EXP_SHIFT)

            # ---- all input DMAs up front. Each ring is ONE serial hw
            # queue (measured ~160GB/s at 2KB lines, ~250-300 at 4KB), so
            # full-d-tile x^T transfers (4KB lines) are split across the
            # sync+scalar rings while gpsimd's software queue (~170GB/s)
            # takes mt and most of x8. uv goes first on sync: the whole
            # projection phase waits on it.
            def xt_full(ring, dt):   # x^T d-tile dt, all 2048 cols (512KB)
                ring.dma_start(out=xt_sb[:, dt, :], in_=xt_d[:, dt, :])

            def x8q(ring, q):    # x8 quad: key tiles 4q..4q+3 (512KB)
                ring.dma_start(out=x8_sb[:, 4 * q:4 * q + 4, :],
                               in_=x8_d[:, 4 * q:4 * q + 4, :])

            def xqp(ring, b):    # xq pair: query blocks b, b+1 (512KB)
                ring.dma_start(out=xq_sb[:, b:b + 2, :],
                               in_=xq_d[:, b:b + 2, :])

            # aggregate HBM read is the wall (~350GB/s across all queues),
            # so rings strictly prioritize: x^T (which gates everything)
            # split across all three, then x8 quads, then xq. Only x8q0
            # jumps the queue (gpsimd, early) so the first delta group
            # isn't gated on the whole x^T load finishing first.
            nc.sync.dma_start(out=uv_sb, in_=uv_d)
            for dt in (0, 2, 4, 6):
                xt_full(nc.sync, dt)
            x8q(nc.sync, 1)
            xqp(nc.sync, 0)
            xqp(nc.sync, 2)
            for dt in (1, 3, 5, 7):
                xt_full(nc.scalar, dt)
            x8q(nc.scalar, 2)
            xqp(nc.scalar, 4)
            xqp(nc.scalar, 6)
            nc.gpsimd.dma_start(out=mt_sb, in_=mt_d)
            x8q(nc.gpsimd, 0)
            nc.gpsimd.dma_start(out=id_sb, in_=id_d)
            x8q(nc.gpsimd, 3)

            # ---- pools ----
            work = ctx.enter_context(tc.tile_pool(name="work", bufs=2))
            keep = ctx.enter_context(tc.tile_pool(name="keep", bufs=1))
            small = ctx.enter_context(tc.tile_pool(name="small", bufs=3))
            rsq_sb = keep.tile([128, NQB], f32)   # softmax rowsums, [q,1]/qb

            # PSUM budget (8 banks): phase 0: warm(1) + ps0(4) + st(1x2) +
            # rr(1) = 8; phase 1 (projections done): st(2) + rr(1) + d(5)
            # = 8; phase 2 (scores+rowsums done): d(6).
            phaseA = ctx.enter_context(contextlib.ExitStack())
            st_pool = phaseA.enter_context(
                tc.tile_pool(name="st_ps", bufs=1, space="PSUM"))
            rr_pool = phaseA.enter_context(
                tc.tile_pool(name="rr_ps", bufs=1, space="PSUM"))

            def st_pair(qc, t):
                """St = Kt_kt^T @ Qt_qc for kt pair (2t, 2t+1); Et = exp."""
                qlo = qc * 512
                st_ps = st_pool.tile([128, 2, 512], f32,
                                     name=f"st_{qc}_{t}", tag="st")
                for h in range(2):
                    kt = 2 * t + h
                    nc.tensor.matmul(
                        st_ps[:, h],
                        kt_sb[:, kt * 128:(kt + 1) * 128],
                        qt_sb[:, qlo:qlo + 512],
                        start=True, stop=True,
                    )
                nc.scalar.activation(out=et_sb[:, qc, t], in_=st_ps,
                                     func=AF.Exp, bias=shift_sb)

            def rs_mm(qc, t, rr_ps):
                """one accumulating DoubleRow step of rs = ones^T @ Et; the
                all-ones stationary is [128, 2, 128] (M=1 fails the walrus
                ldweights ISA check), so every psum partition receives the
                same rowsum row — rs_fix reads row 0."""
                nc.tensor.matmul(
                    rr_ps, ones_sb,
                    et_sb[:, qc, t],
                    start=(t == 0), stop=(t == NKT // 2 - 1),
                    perf_mode=DR,
                )

            def rs_fix(qc, rr_ps):
                """rowsums psum -> sbuf, then layout fix [1,q] -> [q,1] per
                query block via tiny PE transposes sharing the rr bank."""
                rs_sb = small.tile([1, 512], f32, tag="rs_sb")
                nc.vector.tensor_copy(rs_sb, rr_ps[0:1, :])
                for j in range(4):
                    qb = qc * 4 + j
                    nc.tensor.transpose(rr_ps[:, j:j + 1],
                                        rs_sb[0:1, j * 128:(j + 1) * 128],
                                        id_sb[0:1, 0:1])
                    nc.vector.tensor_copy(rsq_sb[:, qb:qb + 1],
                                          rr_ps[:, j:j + 1])

            def epi_half(qb, d_ps, dc, y, zs, zss):
                """z half: rs*x_q + delta for 512 features. sum(z) rides the
                same DVE op via accum_out; sum(z^2) goes to the idle ACT as
                Square+accum."""
                lo, hi = dc * 512, (dc + 1) * 512
                nc.vector.scalar_tensor_tensor(
                    out=y[:, lo:hi], in0=xq_sb[:, qb, lo:hi],
                    scalar=rsq_sb[:, qb:qb + 1], in1=d_ps,
                    op0=mybir.AluOpType.mult, op1=mybir.AluOpType.add,
                    accum_out=zs[:, dc:dc + 1],
                )
                nc.scalar.activation(out=sq_scr, in_=y[:, lo:hi],
                                     func=AF.Square,
                                     accum_out=zss[:, dc:dc + 1])

            def epi_finish(qb, y, zs, zss, ts_act=False):
                """out = LN(z) from the accumulated moments:
                var = (sum(z^2) - sum(z)^2/D) / D; out = z*rstd - mean*rstd.
                Split per dc half so each half's store DMA starts early."""
                t1 = small.tile([128, 1], f32, tag="t1")
                nc.vector.tensor_add(t1, zs[:, 0:1], zs[:, 1:2])
                dv = small.tile([128, 1], f32, tag="dv")
                nc.vector.scalar_tensor_tensor(
                    out=dv, in0=t1, scalar=1.0 / D, in1=t1,
                    op0=mybir.AluOpType.mult, op1=mybir.AluOpType.mult)
                t2 = small.tile([128, 1], f32, tag="t2")
                nc.vector.tensor_add(t2, zss[:, 0:1], zss[:, 1:2])
                vv = small.tile([128, 1], f32, tag="vv")
                nc.vector.tensor_sub(vv, t2, dv)
                sd = small.tile([128, 1], f32, tag="sd")
                nc.scalar.activation(out=sd, in_=vv, func=AF.Sqrt,
                                     scale=1.0 / D, bias=eps_sb)
                rstd = small.tile([128, 1], f32, tag="rstd")
                nc.vector.reciprocal(rstd, sd)
                nmr = small.tile([128, 1], f32, tag="nmr")
                nc.vector.scalar_tensor_tensor(
                    out=nmr, in0=t1, scalar=-1.0 / D, in1=rstd,
                    op0=mybir.AluOpType.mult, op1=mybir.AluOpType.mult)
                o_sb = work.tile([128, D], bf16, tag="o")
                # store halves on alternating rings; for the tail pair the
                # out-scale can run on ACT (Copy table is always resident)
                # so the last two epilogues don't serialize on DVE.
                store_rings = [nc.sync, nc.scalar]
                for dc in range(2):
                    lo, hi = dc * 512, (dc + 1) * 512
                    if ts_act:
                        nc.scalar.activation(out=o_sb[:, lo:hi],
                                             in_=y[:, lo:hi],
                                             func=AF.Identity,
                                             scale=rstd, bias=nmr)
                    else:
                        nc.vector.tensor_scalar(out=o_sb[:, lo:hi],
                                                in0=y[:, lo:hi],
                                                scalar1=rstd, scalar2=nmr,
                                                op0=mybir.AluOpType.mult,
                                                op1=mybir.AluOpType.add)
                    store_rings[dc].dma_start(
                        out=out_d[qb * 128:(qb + 1) * 128, lo:hi],
                        in_=o_sb[:, lo:hi])

            def make_chain(qb, dc, d_pool, y, zs, zss):
                """8 accumulating delta matmul thunks for one (qb, dc) half;
                the stop matmul issues the half's DVE/ACT epilogue inline."""
                qc, j = divmod(qb, 4)
                d_ps = d_pool.tile([128, 512], f32, name=f"d_{qb}_{dc}",
                                   tag="d")
                mms = []
                for t in range(NKT // 2):
                    def mm(t=t):
                        nc.tensor.matmul(
                            d_ps,
                            et_sb[:, qc, t, :, j * 128:(j + 1) * 128],
                            x8_sb[:, 2 * t:2 * t + 2,
                                  dc * 512:(dc + 1) * 512],
                            start=(t == 0), stop=(t == NKT // 2 - 1),
                            perf_mode=DR,
                        )
                        if t == NKT // 2 - 1:
                            epi_half(qb, d_ps, dc, y, zs, zss)
                    mms.append(mm)
                return mms

            def qb_bufs(qb):
                y = work.tile([128, D], f32, tag="y")
                zs = small.tile([128, 2], f32, tag="zs")
                zss = small.tile([128, 2], f32, tag="zss")
                return y, zs, zss

            # ---- phase 0: warmup spin, then all four projection chunk
            # chains in d-tile lockstep. Each full-d-tile DMA arrival
            # unlocks 4 matmuls (one per chunk); the chains all stop at the
            # last arrival, which is the earliest Qt/Kt can exist anyway
            # (the contraction needs every d-tile). ----
            rr0 = rr_pool.tile([128, 512], f32, name="rr_0", tag="rr")
            with tc.tile_pool(name="warm", bufs=1, space="PSUM") as warm, \
                    tc.tile_pool(name="ps0", bufs=1, space="PSUM") as ps0:
                w_ps = warm.tile([128, 256], f32)

                def spin(n):
                    # PE keep-alive between DMA-paced steps: if the PE goes
                    # idle the HAM clock gate drops it to half duty and the
                    # whole downstream stream runs at half clock.
                    for _ in range(n):
                        nc.tensor.matmul(w_ps, warm_sb[:, 0:128], warm_sb,
                                         start=True, stop=True)

                spin(WARMUP_MM)
                qk = [ps0.tile([128, 512], f32, name=f"qk_{c}")
                      for c in range(4)]
                # d-tile order = expected DMA arrival order (rings
                # alternate); spins fill the gap between arrivals
                for i, dt in enumerate((0, 1, 2, 3, 4, 5, 6, 7)):
                    for c in range(4):
                        nc.tensor.matmul(
                            qk[c], uv_sb[:, dt, :],
                            xt_sb[:, dt, c * 512:(c + 1) * 512],
                            start=(i == 0), stop=(i == NDT - 1),
                        )
                    if i < NDT - 1:
                        spin(8)
                for c in range(4):
                    lo, hi = c * 512, (c + 1) * 512
                    if lo < Q:
                        nc.vector.tensor_mul(qt_sb[:, lo:hi],
                                             qk[c][0:R, :], mt_sb[0:R, lo:hi])
                    nc.vector.tensor_mul(kt_sb[:, lo:hi],
                                         qk[c][R:2 * R, :],
                                         mt_sb[R:2 * R, lo:hi])
                st_pair(0, 0)
                spin(12)
                st_pair(0, 1)
                spin(12)

            # ---- phase 1: first delta pair (qb0, qb1) t-major — each Et
            # tile feeds 4 delta matmuls right as its exp lands (the exp
            # stream on ACT is the pacer here) — with all remaining score
            # pairs and the qc0 rowsum chain as fillers. ----
            d5 = phaseA.enter_context(
                tc.tile_pool(name="d_ps", bufs=5, space="PSUM"))
            y0, zs0, zss0 = qb_bufs(0)
            y1, zs1, zss1 = qb_bufs(1)
            a0 = make_chain(0, 0, d5, y0, zs0, zss0)
            a1 = make_chain(0, 1, d5, y0, zs0, zss0)
            b0 = make_chain(1, 0, d5, y1, zs1, zss1)
            b1 = make_chain(1, 1, d5, y1, zs1, zss1)
            rr1 = rr_pool.tile([128, 512], f32, name="rr_1", tag="rr")

            # one score pair per delta group (a second back-to-back pair
            # would stall on the single st psum buffer waiting for the
            # previous exp to drain)
            ST1 = [(0, 2), (0, 3), (0, 4), (0, 5), (0, 6), (0, 7),
                   (1, 0), (1, 1)]
            for t in range(NKT // 2):
                if t == NKT // 2 - 1:
                    rs_mm(0, 7, rr0)
                    rs_fix(0, rr0)
                st_pair(*ST1[t])
                for mm in (a0[t], a1[t], b0[t], b1[t]):
                    mm()
                if t < NKT // 2 - 1:
                    rs_mm(0, t, rr0)
            epi_finish(0, y0, zs0, zss0)
            epi_finish(1, y1, zs1, zss1)

            # ---- phase 2: pair (2,3) chain-major carrying the remaining
            # qc1 score pairs, the qc1 rowsum chain and its transpose ----
            y2, zs2, zss2 = qb_bufs(2)
            y3, zs3, zss3 = qb_bufs(3)
            c20 = make_chain(2, 0, d5, y2, zs2, zss2)
            c30 = make_chain(3, 0, d5, y3, zs3, zss3)
            c21 = make_chain(2, 1, d5, y2, zs2, zss2)
            c31 = make_chain(3, 1, d5, y3, zs3, zss3)
            ST2 = {0: (1, 2), 3: (1, 3), 6: (1, 4), 9: (1, 5), 12: (1, 6),
                   15: (1, 7)}
            RS2 = {2: 0, 5: 1, 8: 2, 11: 3, 14: 4, 17: 5, 20: 6, 23: 7}
            for i, mm in enumerate(c20 + c30 + c21):
                if i in ST2:
                    st_pair(*ST2[i])
                mm()
                if i in RS2:
                    rs_mm(1, RS2[i], rr1)
            rs_fix(1, rr1)
            epi_finish(2, y2, zs2, zss2)
            for mm in c31:
                mm()
            epi_finish(3, y3, zs3, zss3)

            # ---- phase 3: pairs (4,5) and (6,7), 6 PSUM banks, pure
            # deltas; the final pair's out-scales ride ACT so only one
            # short DVE chain trails the last matmul ----
            phaseA.close()
            d6 = ctx.enter_context(
                tc.tile_pool(name="d_ps_b", bufs=6, space="PSUM"))
            for qa in range(4, NQB, 2):
                tail = qa + 1 == NQB - 1
                ya, zsa, zssa = qb_bufs(qa)
                yb, zsb, zssb = qb_bufs(qa + 1)
                ca0 = make_chain(qa, 0, d6, ya, zsa, zssa)
                cb0 = make_chain(qa + 1, 0, d6, yb, zsb, zssb)
                for mm in ca0 + cb0:
                    mm()
                ca1 = make_chain(qa, 1, d6, ya, zsa, zssa)
                for mm in ca1:
                    mm()
                epi_finish(qa, ya, zsa, zssa, ts_act=tail)
                cb1 = make_chain(qa + 1, 1, d6, yb, zsb, zssb)
                for mm in cb1:
                    mm()
                epi_finish(qa + 1, yb, zsb, zssb)

    return nc


def prep_core_inputs(x, mask, U, V):
    """Per-core input dicts (host-side sharding/layout prep)."""
    # [D, 2R] -> [128, NDT, 2R]: partition-major so the device DMA is one
    # contiguous 2KB-per-partition read
    uv = np.ascontiguousarray(
        np.concatenate([U, V], axis=1).astype(BF16)
        .reshape(NDT, 128, 2 * R).transpose(1, 0, 2))
    ident = np.eye(128, dtype=np.float32)
    ins = []
    for c in range(NCORES):
        b, h = divmod(c, 2)
        rot = np.roll(np.arange(N), -h * Q)
        xr = np.ascontiguousarray(x[b][rot])            # [N, D] f32
        mr = np.ascontiguousarray(mask[b][rot])         # [N, R] f32
        s = 1.0 / np.sqrt(np.maximum(mr.sum(axis=1), 1.0))   # [N]
        mq = (mr * s[:, None]).astype(BF16).T           # [R, N]
        mk = mr.astype(BF16).T                          # [R, N]
        xbf = xr.astype(BF16)
        # partition-major dram layouts: [...] -> [128, tiles, free] so each
        # partition's dram bytes are one contiguous run (big DMA lines)
        x8p = np.ascontiguousarray(
            xr.astype(FP8).reshape(NKT, 128, D).transpose(1, 0, 2))
        xtp = np.ascontiguousarray(
            xbf.T.reshape(NDT, 128, N).transpose(1, 0, 2))
        xqp = np.ascontiguousarray(
            xbf[:Q].reshape(NQB, 128, D).transpose(1, 0, 2))
        ins.append({
            "x8": x8p,
            "xt": xtp,
            "xq": xqp,
            "mt": np.ascontiguousarray(np.concatenate([mq, mk], axis=0)),
            "uv": uv,
            "ident": ident,
        })
    return ins


WALRUS_MAX_SEM = 176    # the NEFF exit routine wipes semaphores 0..max in
                        # ~51-per-engine serial chunks (~6us); our program
                        # tops out at sem ~170, so cap the wipe there.


def _patch_walrus_maxsem():
    if not WALRUS_MAX_SEM:
        return
    import concourse.bass_utils as bu

    if getattr(bu, "_asp_walrus_shim", None):
        return
    real = bu.get_walrus_driver()
    shim = f"/tmp/asp_walrus_shim_{WALRUS_MAX_SEM}.sh"
    with open(shim, "w") as f:
        f.write(f'#!/bin/sh\nexec {real} "$@" '
                f'--max-sem-num={WALRUS_MAX_SEM}\n')
    os.chmod(shim, 0o755)
    bu.get_walrus_driver = lambda: shim
    bu._asp_walrus_shim = shim


def run_cores(ins, trace=False, trace_kwargs=None):
    from concourse.bass_utils import run_bass_kernel_spmd

    _patch_walrus_maxsem()
    if "nc" not in _CACHE:
        _CACHE["nc"] = build_program()
    kw = {}
    if trace:
        kw["trace"] = True
        kw.update(trace_kwargs or {})
    return run_bass_kernel_spmd(_CACHE["nc"], ins, list(range(NCORES)), **kw)


def kernel(x, mask, U, V, gamma, beta):
    x = np.asarray(x, dtype=np.float32)
    mask = np.asarray(mask, dtype=np.float32)
    U = np.asarray(U, dtype=np.float32)
    V = np.asarray(V, dtype=np.float32)
    gamma = np.asarray(gamma, dtype=np.float32)
    beta = np.asarray(beta, dtype=np.float32)

    ins = prep_core_inputs(x, mask, U, V)
    res = run_cores(ins)
    out = np.empty((B, N, D), dtype=np.float32)
    for c in range(NCORES):
        b, h = divmod(c, 2)
        out[b, h * Q:(h + 1) * Q] = res.results[c]["out"].astype(np.float32)
    return out * gamma + beta


# revision 37
# speedup vs baseline: 1.0586x; 1.0298x over previous
"""ASP layer (low-rank masked attention + residual layernorm) on 8 TRN2 cores.

Sharding: core c handles batch b = c // 2, query half h = c % 2.
Each core receives x/mask for its batch ROTATED so that its 1024 queries are
rows 0:1024 (keys are just permuted; softmax and delta are invariant to key
order). The device program is identical on all cores (SPMD); only data
differs.

Device math per core (N=2048 keys, Q=1024 queries, D=1024, R=64):
  QtKt = [U|V]^T @ x^T          (PE bf16, fp32 accum; x^T precomputed on host)
  Qt   = QtKt[0:64]   * (mask*s).T   (DVE; s = 1/sqrt(r_eff) folded on host)
  Kt   = QtKt[64:128] * mask.T       (DVE)
  St   = Kt_tile^T @ Qt         (PE; scores TRANSPOSED [k, q] so exp output
                                 is directly the delta stationary)
  Et   = exp(St - 3.5)          (ACT, psum -> sbuf FP8 e4m3; softmax and the
                                 rs-scaled LN are shift-invariant.)
  rs   = ones^T @ Et            (PE fp8 DoubleRow; softmax row sums)
  delta= Et^T @ x8              (PE fp8 DoubleRow, fp32 accum)
  z    = rs*x_q + delta         (DVE; LN is scale-invariant per token)
  out  = LN(z)                  (DVE moments + single ACT sqrt)
gamma/beta are applied on the host.

Schedule (v2): the whole kernel is one software-pipelined PE stream.
  - All input DMAs are issued up front on 4 HWDGE rings (SP/ACT/Pool/DVE)
    in consumption order: uv, mt, ident, x^T c0, c1, x8 kt0-3, c2, kt4-7,
    c3, kt8-15, xq. ~6.75MB of critical bytes => first delta possible ~17us.
  - Warmup spin sized to end when uv+mt+x^T c0 land (opens the PE clock
    gate and hides the initial DMA latency).
  - Projections interleave qc0 scores+rowsums; the first delta pair (qb0/1)
    runs chain-major right after the projections with qc1 scores + qc1
    rowsums + both rowsum transposes as fillers; pairs (2,3),(4,5),(6,7)
    then stream clean with 6 PSUM banks so only qb7's epilogue trails the
    final matmul.
  - Tail: the tile drain's semaphore waits are distributed across PE/ACT/SP
    in parallel (instead of ~11 serial NOPs on SP), DVE/Pool gate on a flag
    semaphore, and Pool alone resets+clears the tile semaphore range. No
    full engine barriers at the end.
"""

import os
import sys

sys.path.insert(0, "/opt/trn_rl_repo")

import numpy as np
import ml_dtypes

B, N, D, R = 4, 2048, 1024, 64
NCORES = 8
Q = N // 2          # queries per core
NQB = Q // 128      # query blocks per core
NKT = N // 128      # key tiles
NDT = D // 128      # d tiles
LN_EPS = 1e-5
WARMUP_MM = 65      # 256-col PE spin sized to end as the first x^T d-tile
                    # lands in SBUF (~12us; full-clock spin mm = ~107ns)
EXP_SHIFT = -3.5    # exp(s + EXP_SHIFT): keeps Et below the e4m3 max of 240

BF16 = ml_dtypes.bfloat16
FP8 = ml_dtypes.float8_e4m3

_CACHE = {}


def _split_waits(nc, max_waits=1):
    """walrus in this container rejects instructions carrying more than ~1
    sem-wait (e.g. Drain/CTRL and the XPOSE DMA encodings). Move excess waits
    onto injected same-engine nops that precede the instruction — engines are
    program-ordered, so semantics are unchanged."""
    from concourse import mybir

    n = 0
    for fn in nc.m.functions:
        for bb in fn.blocks:
            insts = bb.instructions
            new_list = []
            for inst in insts:
                si = inst.sync_info
                waits = list(si.on_wait) if si and si.on_wait else []
                if len(waits) > max_waits:
                    excess = waits[: -max_waits]
                    si.on_wait = waits[-max_waits:]
                    for w in excess:
                        nop = mybir.InstNoOp(name=f"I-wsplit-{n}", ins=[],
                                             outs=[])
                        n += 1
                        nop.engine = inst.engine
                        nop.sync_info = mybir.SyncInfo(on_wait=[w],
                                                       on_update=[])
                        nc.register_instruction(nop)
                        new_list.append(nop)
                new_list.append(inst)
            insts[:] = new_list


def _patch_tile_drain():
    import concourse.tile as tile
    from concourse import mybir
    from concourse.vector_clock import ScopedClock

    if getattr(tile.TileContext, "_drain_patched", False):
        return

    def _drain_and_barrier(self, tick_clock, wait_clock):
        nc = self.nc
        # Collect the full end-of-kernel wait set on the sync drain, then
        # redistribute it: one wait per NOP, round-robin across PE/ACT/SP so
        # the ~600ns-per-sem-check cost is paid in parallel instead of as a
        # serial chain on SP. DVE and Pool (whose end-of-NEFF semaphore-wipe
        # chunks contain the live tile sems) gate on a flag incremented by
        # the three waiting engines; Pool then resets the tile sem range for
        # re-execution. No full engine barriers.
        drain_inst = nc.sync.drain()
        wait_clock.add_sem_waits(
            drain_inst.ins, ScopedClock({None: tick_clock.global_clock})
        )
        assert self.sems is not None
        popped = nc._tile_sem_poison_stack.pop()
        assert popped is self._sem_poison

        si = drain_inst.ins.sync_info
        waits = list(si.on_wait) if si and si.on_wait else []
        si.on_wait = []

        sem_nums = sorted(
            s.num if hasattr(s, "num") else s
            for s in self.sems.allocated().values()
        )
        flag = nc.alloc_semaphore("tail_flag")

        wait_engines = [nc.tensor, nc.scalar, nc.sync]
        for i, w in enumerate(waits):
            eng = wait_engines[i % len(wait_engines)]
            nop = eng.nop()
            nop.ins.sync_info = mybir.SyncInfo(on_wait=[w], on_update=[])
        for eng in wait_engines:
            eng.sem_inc(flag, 1)
        nc.vector.wait_ge(flag, len(wait_engines))
        nc.gpsimd.wait_ge(flag, len(wait_engines))
        if sem_nums:
            lo, hi = min(sem_nums), max(sem_nums)
            rng = range(lo, max(hi, flag.num) + 1)
            nc.gpsimd.dma_reset(rng)
            nc.gpsimd.sem_clear(rng)
        _split_waits(nc)

    tile.TileContext._drain_and_barrier = _drain_and_barrier
    tile.TileContext._drain_patched = True


def build_program():
    import contextlib

    import concourse.bass as bass
    import concourse.tile as tile
    from concourse import mybir

    _patch_tile_drain()
    f32 = mybir.dt.float32
    bf16 = mybir.dt.bfloat16
    fp8 = mybir.dt.float8e4
    AF = mybir.ActivationFunctionType
    DR = mybir.MatmulPerfMode.DoubleRow

    nc = bass.Bass("TRN2", target_bir_lowering=False, debug=False,
                   num_devices=NCORES)

    # x8/xt/xq are stored PARTITION-MAJOR in dram (host pre-shuffle): dram
    # bytes for partition p are contiguous, so every DMA line is a 2-4KB
    # linear run instead of 1KB (descriptor-rate-bound DMA runs ~2-4x
    # faster per queue).
    x8_d = nc.dram_tensor("x8", [128, NKT, D], fp8, kind="ExternalInput").ap()
    xt_d = nc.dram_tensor("xt", [128, NDT, N], bf16,
                          kind="ExternalInput").ap()
    xq_d = nc.dram_tensor("xq", [128, NQB, D], bf16,
                          kind="ExternalInput").ap()
    mt_d = nc.dram_tensor("mt", [2 * R, N], bf16, kind="ExternalInput").ap()
    uv_d = nc.dram_tensor("uv", [128, NDT, 2 * R], bf16,
                          kind="ExternalInput").ap()
    id_d = nc.dram_tensor("ident", [128, 128], f32, kind="ExternalInput").ap()
    out_d = nc.dram_tensor("out", [Q, D], bf16, kind="ExternalOutput").ap()

    with tile.TileContext(nc) as tc:
        with contextlib.ExitStack() as ctx:
            const = ctx.enter_context(tc.tile_pool(name="const", bufs=1))
            eps_sb = const.tile([128, 1], f32)
            shift_sb = const.tile([128, 1], f32)
            ones_sb = const.tile([128, 2, 128], fp8)
            warm_sb = const.tile([128, 256], bf16)
            uv_sb = const.tile([128, NDT, 2 * R], bf16)
            xt_sb = const.tile([128, NDT, N], bf16)
            mt_sb = const.tile([2 * R, N], bf16)
            x8_sb = const.tile([128, NKT, D], fp8)
            xq_sb = const.tile([128, NQB, D], bf16)
            id_sb = const.tile([128, 128], f32)
            qt_sb = const.tile([R, Q], bf16)
            kt_sb = const.tile([R, N], bf16)
            # Et layout: [p, qc, t(=kt pair), h, 512] — h indexes the kt pair
            # so [:, qc, t] is a ready-made [128, 2, 512] DoubleRow operand
            et_sb = const.tile([128, 2, NKT // 2, 2, 512], fp8)
            # throwaway Square output (only its accum_out matters); same-
            # engine WAW ordering makes sharing one buffer safe
            sq_scr = const.tile([128, 512], f32)

            # warm_sb first: the PE warmup spin waits only on this memset
            nc.vector.memset(warm_sb, 0.5)
            nc.vector.memset(ones_sb, 1.0)
            nc.vector.memset(eps_sb, LN_EPS)
            nc.vector.memset(shift_sb, EXP_SHIFT)

            # ---- all input DMAs up front. Each ring is ONE serial hw
            # queue (measured ~160GB/s at 2KB lines, ~250-300 at 4KB), so
            # full-d-tile x^T transfers (4KB lines) are split across the
            # sync+scalar rings while gpsimd's software queue (~170GB/s)
            # takes mt and most of x8. uv goes first on sync: the whole
            # projection phase waits on it.
            def xt_full(ring, dt):   # x^T d-tile dt, all 2048 cols (512KB)
                ring.dma_start(out=xt_sb[:, dt, :], in_=xt_d[:, dt, :])

            def x8q(ring, q):    # x8 quad: key tiles 4q..4q+3 (512KB)
                ring.dma_start(out=x8_sb[:, 4 * q:4 * q + 4, :],
                               in_=x8_d[:, 4 * q:4 * q + 4, :])

            def xqp(ring, b):    # xq pair: query blocks b, b+1 (512KB)
                ring.dma_start(out=xq_sb[:, b:b + 2, :],
                               in_=xq_d[:, b:b + 2, :])

            # aggregate HBM read is the wall (~350GB/s across all queues),
            # so rings strictly prioritize: x^T (which gates everything)
            # split across all three, then x8 quads, then xq. Only x8q0
            # jumps the queue (gpsimd, early) so the first delta group
            # isn't gated on the whole x^T load finishing first.
            nc.sync.dma_start(out=uv_sb, in_=uv_d)
            for dt in (0, 2, 4, 6):
                xt_full(nc.sync, dt)
            x8q(nc.sync, 1)
            xqp(nc.sync, 0)
            xqp(nc.sync, 2)
            for dt in (1, 3, 5, 7):
                xt_full(nc.scalar, dt)
            x8q(nc.scalar, 2)
            xqp(nc.scalar, 4)
            xqp(nc.scalar, 6)
            nc.gpsimd.dma_start(out=mt_sb, in_=mt_d)
            x8q(nc.gpsimd, 0)
            nc.gpsimd.dma_start(out=id_sb, in_=id_d)
            x8q(nc.gpsimd, 3)

            # ---- pools ----
            work = ctx.enter_context(tc.tile_pool(name="work", bufs=2))
            keep = ctx.enter_context(tc.tile_pool(name="keep", bufs=1))
            small = ctx.enter_context(tc.tile_pool(name="small", bufs=3))
            rsq_sb = keep.tile([128, NQB], f32)   # softmax rowsums, [q,1]/qb

            # PSUM budget (8 banks): phase 0: warm(1) + ps0(4) + st(1x2) +
            # rr(1) = 8; phase 1 (projections done): st(2) + rr(1) + d(5)
            # = 8; phase 2 (scores+rowsums done): d(6).
            phaseA = ctx.enter_context(contextlib.ExitStack())
            st_pool = phaseA.enter_context(
                tc.tile_pool(name="st_ps", bufs=2, space="PSUM"))
            rr_pool = phaseA.enter_context(
                tc.tile_pool(name="rr_ps", bufs=1, space="PSUM"))

            def st_pair(qc, t):
                """St = Kt_kt^T @ Qt_qc for kt pair (2t, 2t+1); Et = exp.
                Split into two query-halves so each st tile is ONE psum
                bank: with bufs=2 the next half's matmuls overlap the
                previous half's exp (a full-size tile would serialize
                matmul->exp->matmul at ~2us per pair)."""
                qlo = qc * 512
                for hq in range(2):
                    st_ps = st_pool.tile([128, 2, 256], f32,
                                         name=f"st_{qc}_{t}_{hq}", tag="st")
                    for h in range(2):
                        kt = 2 * t + h
                        nc.tensor.matmul(
                            st_ps[:, h],
                            kt_sb[:, kt * 128:(kt + 1) * 128],
                            qt_sb[:, qlo + hq * 256:qlo + (hq + 1) * 256],
                            start=True, stop=True,
                        )
                    nc.scalar.activation(
                        out=et_sb[:, qc, t, :, hq * 256:(hq + 1) * 256],
                        in_=st_ps, func=AF.Exp, bias=shift_sb)

            def rs_mm(qc, t, rr_ps):
                """one accumulating DoubleRow step of rs = ones^T @ Et; the
                all-ones stationary is [128, 2, 128] (M=1 fails the walrus
                ldweights ISA check), so every psum partition receives the
                same rowsum row — rs_fix reads row 0."""
                nc.tensor.matmul(
                    rr_ps, ones_sb,
                    et_sb[:, qc, t],
                    start=(t == 0), stop=(t == NKT // 2 - 1),
                    perf_mode=DR,
                )

            def rs_fix(qc, rr_ps):
                """rowsums psum -> sbuf, then layout fix [1,q] -> [q,1] per
                query block via tiny PE transposes sharing the rr bank."""
                rs_sb = small.tile([1, 512], f32, tag="rs_sb")
                nc.vector.tensor_copy(rs_sb, rr_ps[0:1, :])
                for j in range(4):
                    qb = qc * 4 + j
                    nc.tensor.transpose(rr_ps[:, j:j + 1],
                                        rs_sb[0:1, j * 128:(j + 1) * 128],
                                        id_sb[0:1, 0:1])
                    nc.vector.tensor_copy(rsq_sb[:, qb:qb + 1],
                                          rr_ps[:, j:j + 1])

            def epi_half(qb, d_ps, dc, y, zs, zss):
                """z half: rs*x_q + delta for 512 features. sum(z) rides the
                same DVE op via accum_out; sum(z^2) goes to the idle ACT as
                Square+accum."""
                lo, hi = dc * 512, (dc + 1) * 512
                nc.vector.scalar_tensor_tensor(
                    out=y[:, lo:hi], in0=xq_sb[:, qb, lo:hi],
                    scalar=rsq_sb[:, qb:qb + 1], in1=d_ps,
                    op0=mybir.AluOpType.mult, op1=mybir.AluOpType.add,
                    accum_out=zs[:, dc:dc + 1],
                )
                nc.scalar.activation(out=sq_scr, in_=y[:, lo:hi],
                                     func=AF.Square,
                                     accum_out=zss[:, dc:dc + 1])

            def epi_finish(qb, y, zs, zss, ts_act=False):
                """out = LN(z) from the accumulated moments:
                var = (sum(z^2) - sum(z)^2/D) / D; out = z*rstd - mean*rstd.
                Split per dc half so each half's store DMA starts early."""
                t1 = small.tile([128, 1], f32, tag="t1")
                nc.vector.tensor_add(t1, zs[:, 0:1], zs[:, 1:2])
                dv = small.tile([128, 1], f32, tag="dv")
                nc.vector.scalar_tensor_tensor(
                    out=dv, in0=t1, scalar=1.0 / D, in1=t1,
                    op0=mybir.AluOpType.mult, op1=mybir.AluOpType.mult)
                t2 = small.tile([128, 1], f32, tag="t2")
                nc.vector.tensor_add(t2, zss[:, 0:1], zss[:, 1:2])
                vv = small.tile([128, 1], f32, tag="vv")
                nc.vector.tensor_sub(vv, t2, dv)
                sd = small.tile([128, 1], f32, tag="sd")
                nc.scalar.activation(out=sd, in_=vv, func=AF.Sqrt,
                                     scale=1.0 / D, bias=eps_sb)
                rstd = small.tile([128, 1], f32, tag="rstd")
                nc.vector.reciprocal(rstd, sd)
                nmr = small.tile([128, 1], f32, tag="nmr")
                nc.vector.scalar_tensor_tensor(
                    out=nmr, in0=t1, scalar=-1.0 / D, in1=rstd,
                    op0=mybir.AluOpType.mult, op1=mybir.AluOpType.mult)
                o_sb = work.tile([128, D], bf16, tag="o")
                # store halves on alternating rings; for the tail pair the
                # out-scale can run on ACT (Copy table is always resident)
                # so the last two epilogues don't serialize on DVE.
                store_rings = [nc.sync, nc.scalar]
                for dc in range(2):
                    lo, hi = dc * 512, (dc + 1) * 512
                    if ts_act:
                        nc.scalar.activation(out=o_sb[:, lo:hi],
                                             in_=y[:, lo:hi],
                                             func=AF.Identity,
                                             scale=rstd, bias=nmr)
                    else:
                        nc.vector.tensor_scalar(out=o_sb[:, lo:hi],
                                                in0=y[:, lo:hi],
                                                scalar1=rstd, scalar2=nmr,
                                                op0=mybir.AluOpType.mult,
                                                op1=mybir.AluOpType.add)
                    store_rings[dc].dma_start(
                        out=out_d[qb * 128:(qb + 1) * 128, lo:hi],
                        in_=o_sb[:, lo:hi])

            def make_chain(qb, dc, d_pool, y, zs, zss):
                """8 accumulating delta matmul thunks for one (qb, dc) half;
                the stop matmul issues the half's DVE/ACT epilogue inline."""
                qc, j = divmod(qb, 4)
                d_ps = d_pool.tile([128, 512], f32, name=f"d_{qb}_{dc}",
                                   tag="d")
                mms = []
                for t in range(NKT // 2):
                    def mm(t=t):
                        nc.tensor.matmul(
                            d_ps,
                            et_sb[:, qc, t, :, j * 128:(j + 1) * 128],
                            x8_sb[:, 2 * t:2 * t + 2,
                                  dc * 512:(dc + 1) * 512],
                            start=(t == 0), stop=(t == NKT // 2 - 1),
                            perf_mode=DR,
                        )
                        if t == NKT // 2 - 1:
                            epi_half(qb, d_ps, dc, y, zs, zss)
                    mms.append(mm)
                return mms

            def qb_bufs(qb):
                y = work.tile([128, D], f32, tag="y")
                zs = small.tile([128, 2], f32, tag="zs")
                zss = small.tile([128, 2], f32, tag="zss")
                return y, zs, zss

            # ---- phase 0: warmup spin, then all four projection chunk
            # chains in d-tile lockstep. Each full-d-tile DMA arrival
            # unlocks 4 matmuls (one per chunk); the chains all stop at the
            # last arrival, which is the earliest Qt/Kt can exist anyway
            # (the contraction needs every d-tile). ----
            rr0 = rr_pool.tile([128, 512], f32, name="rr_0", tag="rr")
            with tc.tile_pool(name="warm", bufs=1, space="PSUM") as warm, \
                    tc.tile_pool(name="ps0", bufs=1, space="PSUM") as ps0:
                w_ps = warm.tile([128, 256], f32)

                def spin(n):
                    # PE keep-alive between DMA-paced steps: if the PE goes
                    # idle the HAM clock gate drops it to half duty and the
                    # whole downstream stream runs at half clock.
                    for _ in range(n):
                        nc.tensor.matmul(w_ps, warm_sb[:, 0:128], warm_sb,
                                         start=True, stop=True)

                spin(WARMUP_MM)
                qk = [ps0.tile([128, 512], f32, name=f"qk_{c}")
                      for c in range(4)]
                # d-tile order = expected DMA arrival order (rings
                # alternate); spins fill the gap between arrivals
                for i, dt in enumerate((0, 1, 2, 3, 4, 5, 6, 7)):
                    for c in range(4):
                        nc.tensor.matmul(
                            qk[c], uv_sb[:, dt, :],
                            xt_sb[:, dt, c * 512:(c + 1) * 512],
                            start=(i == 0), stop=(i == NDT - 1),
                        )
                    if i < NDT - 1:
                        spin(8)
                for c in range(4):
                    lo, hi = c * 512, (c + 1) * 512
                    if lo < Q:
                        nc.vector.tensor_mul(qt_sb[:, lo:hi],
                                             qk[c][0:R, :], mt_sb[0:R, lo:hi])
                    nc.vector.tensor_mul(kt_sb[:, lo:hi],
                                         qk[c][R:2 * R, :],
                                         mt_sb[R:2 * R, lo:hi])
                st_pair(0, 0)
                spin(12)
                st_pair(0, 1)
                spin(12)

            # ---- phase 1: first delta pair (qb0, qb1) t-major — each Et
            # tile feeds 4 delta matmuls right as its exp lands (the exp
            # stream on ACT is the pacer here) — with all remaining score
            # pairs and the qc0 rowsum chain as fillers. ----
            d5 = phaseA.enter_context(
                tc.tile_pool(name="d_ps", bufs=5, space="PSUM"))
            y0, zs0, zss0 = qb_bufs(0)
            y1, zs1, zss1 = qb_bufs(1)
            a0 = make_chain(0, 0, d5, y0, zs0, zss0)
            a1 = make_chain(0, 1, d5, y0, zs0, zss0)
            b0 = make_chain(1, 0, d5, y1, zs1, zss1)
            b1 = make_chain(1, 1, d5, y1, zs1, zss1)
            rr1 = rr_pool.tile([128, 512], f32, name="rr_1", tag="rr")

            # one score pair per delta group (a second back-to-back pair
            # would stall on the single st psum buffer waiting for the
            # previous exp to drain)
            ST1 = [(0, 2), (0, 3), (0, 4), (0, 5), (0, 6), (0, 7),
                   (1, 0), (1, 1)]
            for t in range(NKT // 2):
                if t == NKT // 2 - 1:
                    rs_mm(0, 7, rr0)
                    rs_fix(0, rr0)
                st_pair(*ST1[t])
                for mm in (a0[t], a1[t], b0[t], b1[t]):
                    mm()
                if t < NKT // 2 - 1:
                    rs_mm(0, t, rr0)
            epi_finish(0, y0, zs0, zss0)
            epi_finish(1, y1, zs1, zss1)

            # ---- phase 2: pair (2,3) chain-major carrying the remaining
            # qc1 score pairs, the qc1 rowsum chain and its transpose ----
            y2, zs2, zss2 = qb_bufs(2)
            y3, zs3, zss3 = qb_bufs(3)
            c20 = make_chain(2, 0, d5, y2, zs2, zss2)
            c30 = make_chain(3, 0, d5, y3, zs3, zss3)
            c21 = make_chain(2, 1, d5, y2, zs2, zss2)
            c31 = make_chain(3, 1, d5, y3, zs3, zss3)
            ST2 = {0: (1, 2), 3: (1, 3), 6: (1, 4), 9: (1, 5), 12: (1, 6),
                   15: (1, 7)}
            RS2 = {2: 0, 5: 1, 8: 2, 11: 3, 14: 4, 17: 5, 20: 6, 23: 7}
            for i, mm in enumerate(c20 + c30 + c21):
                if i in ST2:
                    st_pair(*ST2[i])
                mm()
                if i in RS2:
                    rs_mm(1, RS2[i], rr1)
            rs_fix(1, rr1)
            epi_finish(2, y2, zs2, zss2)
            for mm in c31:
                mm()
            epi_finish(3, y3, zs3, zss3)

            # ---- phase 3: pairs (4,5) and (6,7), 6 PSUM banks, pure
            # deltas; the final pair's out-scales ride ACT so only one
            # short DVE chain trails the last matmul ----
            phaseA.close()
            d6 = ctx.enter_context(
                tc.tile_pool(name="d_ps_b", bufs=6, space="PSUM"))
            for qa in range(4, NQB, 2):
                tail = qa + 1 == NQB - 1
                ya, zsa, zssa = qb_bufs(qa)
                yb, zsb, zssb = qb_bufs(qa + 1)
                ca0 = make_chain(qa, 0, d6, ya, zsa, zssa)
                cb0 = make_chain(qa + 1, 0, d6, yb, zsb, zssb)
                for mm in ca0 + cb0:
                    mm()
                ca1 = make_chain(qa, 1, d6, ya, zsa, zssa)
                for mm in ca1:
                    mm()
                epi_finish(qa, ya, zsa, zssa, ts_act=tail)
                cb1 = make_chain(qa + 1, 1, d6, yb, zsb, zssb)
                for mm in cb1:
                    mm()
                epi_finish(qa + 1, yb, zsb, zssb)

    return nc


def prep_core_inputs(x, mask, U, V):
    """Per-core input dicts (host-side sharding/layout prep)."""
    # [D, 2R] -> [128, NDT, 2R]: partition-major so the device DMA is one
    # contiguous 2KB-per-partition read
    uv = np.ascontiguousarray(
        np.concatenate([U, V], axis=1).astype(BF16)
        .reshape(NDT, 128, 2 * R).transpose(1, 0, 2))
    ident = np.eye(128, dtype=np.float32)
    ins = []
    for c in range(NCORES):
        b, h = divmod(c, 2)
        rot = np.roll(np.arange(N), -h * Q)
        xr = np.ascontiguousarray(x[b][rot])            # [N, D] f32
        mr = np.ascontiguousarray(mask[b][rot])         # [N, R] f32
        s = 1.0 / np.sqrt(np.maximum(mr.sum(axis=1), 1.0))   # [N]
        mq = (mr * s[:, None]).astype(BF16).T           # [R, N]
        mk = mr.astype(BF16).T                          # [R, N]
        xbf = xr.astype(BF16)
        # partition-major dram layouts: [...] -> [128, tiles, free] so each
        # partition's dram bytes are one contiguous run (big DMA lines)
        x8p = np.ascontiguousarray(
            xr.astype(FP8).reshape(NKT, 128, D).transpose(1, 0, 2))
        xtp = np.ascontiguousarray(
            xbf.T.reshape(NDT, 128, N).transpose(1, 0, 2))
        xqp = np.ascontiguousarray(
            xbf[:Q].reshape(NQB, 128, D).transpose(1, 0, 2))
        ins.append({
            "x8": x8p,
            "xt": xtp,
            "xq": xqp,
            "mt": np.ascontiguousarray(np.concatenate([mq, mk], axis=0)),
            "uv": uv,
            "ident": ident,
        })
    return ins


WALRUS_MAX_SEM = 176    # the NEFF exit routine wipes semaphores 0..max in
                        # ~51-per-engine serial chunks (~6us); our program
                        # tops out at sem ~170, so cap the wipe there.


def _patch_walrus_maxsem():
    if not WALRUS_MAX_SEM:
        return
    import concourse.bass_utils as bu

    if getattr(bu, "_asp_walrus_shim", None):
        return
    real = bu.get_walrus_driver()
    shim = f"/tmp/asp_walrus_shim_{WALRUS_MAX_SEM}.sh"
    with open(shim, "w") as f:
        f.write(f'#!/bin/sh\nexec {real} "$@" '
                f'--max-sem-num={WALRUS_MAX_SEM}\n')
    os.chmod(shim, 0o755)
    bu.get_walrus_driver = lambda: shim
    bu._asp_walrus_shim = shim


def run_cores(ins, trace=False, trace_kwargs=None):
    from concourse.bass_utils import run_bass_kernel_spmd

    _patch_walrus_maxsem()
    if "nc" not in _CACHE:
        _CACHE["nc"] = build_program()
    kw = {}
    if trace:
        kw["trace"] = True
        kw.update(trace_kwargs or {})
    return run_bass_kernel_spmd(_CACHE["nc"], ins, list(range(NCORES)), **kw)


def kernel(x, mask, U, V, gamma, beta):
    x = np.asarray(x, dtype=np.float32)
    mask = np.asarray(mask, dtype=np.float32)
    U = np.asarray(U, dtype=np.float32)
    V = np.asarray(V, dtype=np.float32)
    gamma = np.asarray(gamma, dtype=np.float32)
    beta = np.asarray(beta, dtype=np.float32)

    ins = prep_core_inputs(x, mask, U, V)
    res = run_cores(ins)
    out = np.empty((B, N, D), dtype=np.float32)
    for c in range(NCORES):
        b, h = divmod(c, 2)
        out[b, h * Q:(h + 1) * Q] = res.results[c]["out"].astype(np.float32)
    return out * gamma + beta
